# revision 1
# baseline (speedup 1.0000x reference)
"""TRN2 Bass kernel for nn_Attn_Pred_Model (sparse_attention, memory-bound).

Computes, per (batch, head) slice of x [S=4096, B=64]:
    out[s] = (sum_{i=0..7} alpha*beta^i * x[s-i-1]  + pb_fwd + pb_bwd[arange2]) * mask

Strategy (8 NeuronCores, data-parallel over the 256 batch*head slices, 32/core):
  - Rows on partitions, 2 consecutive rows per partition -> 512B DMA
    descriptor runs at full 128 partitions on BOTH loads and stores
    (a 124-partition store measured 3.3x slower). Each slice is 16 windows
    of 256 rows; window w partition k holds rows 256w+2k, 256w+2k+1 -- a
    pure reshape, so x needs no host copy at all.
  - The 8-tap causal shift-sum becomes 4 stationary band matrices on the
    TensorEngine: W[j,j'][k,m] = alpha*beta^(d-1), d = 2(m-k)+j-j' in
    [1,8]; psum[j half] accumulates W[j,j']^T @ x[w] over j'.
  - Host folds biases: biasm = (pb_fwd + pb_bwd[arange2]) * mask; device
    evacuates PSUM as psum*mask + biasm (DVE), mask/biasm resident.
  - The first 8 rows of every 256-row window lack their cross-window taps
    (in rows < window start); host computes those 16*8=128 rows per slice
    exactly (vectorized 8-tap FIR, ~3% of rows) and patches them into the
    gathered output.
  - Loads/stores split across the two HWDGE rings (SP + ACT), alternating
    per slice.
"""

import numpy as np

import concourse.bacc as bacc
import concourse.mybir as mybir
from concourse.bass import AP
from concourse.tile import TileContext
from concourse.bass_utils import run_bass_kernel_spmd

S = 4096            # rows per slice
B = 64              # buckets (free dim)
NCORES = 8
NSL = 32            # slices per core (16*16/8)
WIN = 16            # windows per slice
WROW = 256 * B      # elements per window (16384)
CHUNK = 4           # windows per psum chunk

_CACHE = {}


def _build_nc(loop_n=1):
    nc = bacc.Bacc(None, name="attnpred", enable_partition_id=False)
    f32 = mybir.dt.float32
    x = nc.dram_tensor("x", [NSL * S, B], f32, kind="ExternalInput")
    w = nc.dram_tensor("w", [4, 128, 128], f32, kind="ExternalInput")
    mask = nc.dram_tensor("mask", [S, B], f32, kind="ExternalInput")
    biasm = nc.dram_tensor("biasm", [S, B], f32, kind="ExternalInput")
    y = nc.dram_tensor("y", [NSL * S, B], f32, kind="ExternalOutput")

    with TileContext(nc) as tc:
        with (
            tc.tile_pool(name="aux", bufs=1) as aux,
            tc.tile_pool(name="xin", bufs=4) as xin,
            tc.tile_pool(name="out", bufs=4) as outp,
            tc.tile_pool(name="ps", bufs=8, space="PSUM") as psp,
        ):
            # resident: W [128, 4*128], mask/biasm in (w, j, b) window layout
            w_sb = aux.tile([128, 4 * 128], f32)
            nc.sync.dma_start(
                out=w_sb.rearrange("k (p m) -> k p m", m=128),
                in_=AP(w, 0, [[128, 128], [128 * 128, 4], [1, 128]]),
            )
            mask_sb = aux.tile([128, WIN * 128], f32)
            biasm_sb = aux.tile([128, WIN * 128], f32)
            for dram, sb in ((mask, mask_sb), (biasm, biasm_sb)):
                nc.sync.dma_start(
                    out=sb.rearrange("m (w jb) -> m w jb", jb=128),
                    in_=AP(dram, 0, [[128, 128], [WROW, WIN], [1, 128]]),
                )

            def body(iv=None):
                for s in range(NSL):
                    ld = nc.sync if s % 2 == 0 else nc.scalar
                    st = nc.scalar if s % 2 == 0 else nc.sync
                    x_sb = xin.tile([128, WIN * 128], f32, tag="x")
                    ld.dma_start(
                        out=x_sb.rearrange("k (w jb) -> k w jb", jb=128),
                        in_=AP(x, s * S * B, [[128, 128], [WROW, WIN], [1, 128]]),
                    )
                    x4 = x_sb.rearrange("k (w j b) -> k w j b", j=2, b=B)
                    o_sb = outp.tile([128, WIN * 128], f32, tag="o")
                    o4 = o_sb.rearrange("m (w j b) -> m w j b", j=2, b=B)
                    m4 = mask_sb.rearrange("m (w j b) -> m w j b", j=2, b=B)
                    for w0 in range(0, WIN, CHUNK):
                        nw = CHUNK
                        ps = psp.tile([128, 2 * nw * B], f32, tag="ps")
                        for j in (0, 1):
                            for jp in (0, 1):
                                nc.tensor.matmul(
                                    ps[:, j * nw * B:(j + 1) * nw * B],
                                    w_sb[:, (2 * j + jp) * 128:(2 * j + jp + 1) * 128],
                                    x4[:, w0:w0 + nw, jp, :],
                                    start=(jp == 0),
                                    stop=(jp == 1),
                                )
                        p4 = ps[:, :2 * nw * B].rearrange(
                            "m (j w b) -> m w j b", j=2, b=B)
                        nc.vector.tensor_mul(
                            out=o4[:, w0:w0 + nw],
                            in0=p4,
                            in1=m4[:, w0:w0 + nw],
                        )
                        nc.vector.tensor_add(
                            out=o_sb[:, w0 * 128:(w0 + nw) * 128],
                            in0=o_sb[:, w0 * 128:(w0 + nw) * 128],
                            in1=biasm_sb[:, w0 * 128:(w0 + nw) * 128],
                        )
                    st.dma_start(
                        out=AP(y, s * S * B, [[128, 128], [WROW, WIN], [1, 128]]),
                        in_=o_sb.rearrange("m (w jb) -> m w jb", jb=128),
                    )

            if loop_n == 1:
                body()
            else:
                with tc.For_i(0, loop_n, 1) as iv:
                    body(iv)
    nc.finalize()
    return nc


def _host_prep(x, pb_fwd, pb_bwd, alpha, beta, arange2, mask):
    x = np.ascontiguousarray(np.asarray(x, dtype=np.float32))
    pb_fwd = np.asarray(pb_fwd, dtype=np.float32)
    pb_bwd = np.asarray(pb_bwd, dtype=np.float32)
    alpha = float(np.asarray(alpha).reshape(-1)[0])
    beta = float(np.asarray(beta).reshape(-1)[0])
    arange2 = np.asarray(arange2)
    mask = np.ascontiguousarray(np.asarray(mask, dtype=np.float32))

    c = (alpha * beta ** np.arange(8)).astype(np.float32)
    kk = np.arange(128)[:, None]
    mm = np.arange(128)[None, :]
    w4 = np.zeros((4, 128, 128), np.float32)
    for j in (0, 1):
        for jp in (0, 1):
            d = 2 * (mm - kk) + j - jp
            sel = (d >= 1) & (d <= 8)
            w4[2 * j + jp] = c[np.clip(d, 1, 8) - 1] * sel

    bias = (pb_fwd[0][None, :] + pb_bwd[0][arange2]).astype(np.float32)
    biasm = np.ascontiguousarray(bias * mask)

    xf = x.reshape(NCORES, NSL * S, B)
    in_maps = [
        {"x": xf[core], "w": w4, "mask": mask, "biasm": biasm}
        for core in range(NCORES)
    ]

    # host-exact rows: first 8 rows of each 256-row window (missing
    # cross-window taps on device). patch[sl, w, r] for r in [0,8).
    xs = x.reshape(256, S, B)
    pidx = (256 * np.arange(WIN)[:, None] + np.arange(8)[None, :]).ravel()
    patch = np.zeros((256, len(pidx), B), np.float32)
    for i in range(8):
        src = pidx - 1 - i
        valid = src >= 0
        patch[:, valid] += c[i] * xs[:, src[valid]]
    patch = (patch + bias[pidx]) * mask[pidx]
    return in_maps, (pidx, patch)


def _gather(results, patch_info, out_shape):
    pidx, patch = patch_info
    out = np.empty((NCORES, NSL * S, B), np.float32)
    for core in range(NCORES):
        out[core] = np.asarray(results[core]["y"])
    out = out.reshape(256, S, B)
    out[:, pidx] = patch
    return out.reshape(out_shape)


def kernel(x, pb_fwd, pb_bwd, alpha, beta, arange2, mask):
    in_maps, patch_info = _host_prep(x, pb_fwd, pb_bwd, alpha, beta, arange2, mask)
    if "nc" not in _CACHE:
        _CACHE["nc"] = _build_nc()
    res = run_bass_kernel_spmd(_CACHE["nc"], in_maps, core_ids=list(range(NCORES)))
    return _gather(res.results, patch_info, np.asarray(x).shape)
